# revision 49
# baseline (speedup 1.0000x reference)
"""NetVLAD Trainium2 Bass kernel.

Full-input contract: kernel(**inputs) takes the complete unsharded inputs
(x [32,128,64,64], conv_w [64,128], conv_b [64] (zeros), centroids [64,128])
and returns (global_fea [32,2048], vlad [32,64,128]) like the reference.

Sharding: data-parallel over N across 8 NeuronCores (4 samples per core);
params replicated; no cross-core communication.

x streams to each core in fp16 (11-bit mantissa, ~1e-4 quantization; the
whole-kernel output error vs the fp32 reference measures ~3e-5) in two
layouts so the PE never needs an on-chip transpose and no PSUM->SBUF copies
exist anywhere:
  - channel-major [C, P], the stationary operand of the 1x1-conv matmuls
  - a host-built "vlad rhs image": 129-wide slots per 128-pixel tile
    [XT_tile (pix x chan) | 1/denom column], DMA'd directly into the VLAD
    moving-operand buffer

Per sample:
  - per-tile DVE bn_stats on the pixel-major tiles -> per-pixel sum(x^2) ->
    one Ln + one Exp(scale=-0.5) per sample give rnorm = 1/||x||_2 (both in
    the natural_log_exp table set: exactly one ACT table load, see
    _patch_act_tables)
  - per 8-tile step: 8 logits matmuls into one PSUM bank, then per-tile
    ScalarE exp with per-partition scale=rnorm[tile]: e = exp(rnorm*logit)
    (softmax max-subtraction is skipped: |logit| <= max_k||w_k||_2 ~ 2.5)
  - one grouped DVE reduce gives the softmax denominators; a_hat =
    e * (rnorm/denom); both halves of lhsT = [a_hat | e] land in one tile
  - VLAD accumulated on PE across the 32 tiles of a sample:
      out[0:64, 0:128] = sum_p a_hat[p,k] * XT[p,c]   (vlad numerator)
      out[64:128, 128] = sum_p e[p,k] / denom[p] = s[k]  (softmax mass)
    The post-exp vector ops and vlad matmuls are software-pipelined one
    step behind the exp stage so no engine queue round-trips.
  - host finishes with exact fp32 ops: vlad = vladraw - s[:,None]*centroids,
    then the AdaptiveMaxPool2d((1,2048)) = max over K + 16x column repeat.
"""

import sys

sys.path.insert(0, "/opt/trn_rl_repo")

from contextlib import ExitStack

import numpy as np

import concourse.bacc as bacc
import concourse.tile as tile
from concourse import mybir
from concourse.bass_utils import run_bass_kernel_spmd

F32 = mybir.dt.float32
F32R = mybir.dt.float32r
F16 = mybir.dt.float16

N, C, HW_P, K = 32, 128, 4096, 64
NCORES = 8
S = N // NCORES            # samples per core
TILE = 128                 # pixels per tile
NTILES = HW_P // TILE      # 32
GST = 8                    # tiles per step (one PSUM bank of logits)
NSTEP = NTILES // GST      # 4
VW = TILE + 1              # per-tile slot width in the vlad rhs (129)
RVW = NTILES * VW          # vlad rhs width per sample

_CACHE = {}


def _patch_act_tables():
    """Force every activation into natural_log_exp_and_others (covers Exp,
    Ln, Copy, Identity) so exactly one ACT table load is emitted instead of
    thrashing between exp_and_others and natural_log_exp_and_others."""
    if _CACHE.get("act_patched"):
        return
    orig = bacc.get_activation_tables
    A = mybir.ActivationFunctionType
    strip = {A.Exp, A.Ln, A.Copy, A.Identity}
    keep = "natural_log_exp_and_others"

    def patched(arch):
        t = dict(orig(arch))
        if keep not in t:
            return t
        return {k: (v if k == keep else (set(v) - strip)) for k, v in t.items()}

    bacc.get_activation_tables = patched
    _CACHE["act_patched"] = True


def _build_nc(S=S):
    _patch_act_tables()
    nc = bacc.Bacc(
        "TRN2",
        target_bir_lowering=False,
        debug=False,
        num_devices=NCORES,
    )
    x_d = nc.dram_tensor("x", [S, C, HW_P], F16, kind="ExternalInput")
    # host-prepared vlad-rhs image: 129-wide slots [XT_tile | 0] plus a zero
    # tail, where xt2[n, p, t*129+c] = x[n, c, t*128+p] for c < 128
    xt_d = nc.dram_tensor("xt2", [S, TILE, RVW], F16, kind="ExternalInput")
    wt_d = nc.dram_tensor("wt", [C, K], F16, kind="ExternalInput")
    vladraw_d = nc.dram_tensor("vladraw", [S, K, C], F32, kind="ExternalOutput")
    svec_d = nc.dram_tensor("svec", [S, K, 1], F32, kind="ExternalOutput")

    EXP = mybir.ActivationFunctionType.Exp
    LN = mybir.ActivationFunctionType.Ln
    MUL = mybir.AluOpType.mult
    ADD = mybir.AluOpType.add

    with tile.TileContext(nc) as tc, ExitStack() as ctx:
        xpool = ctx.enter_context(tc.tile_pool(name="xpool", bufs=3))
        rvp = ctx.enter_context(tc.tile_pool(name="rvp", bufs=3))
        consts = ctx.enter_context(tc.tile_pool(name="consts", bufs=1))
        egp = ctx.enter_context(tc.tile_pool(name="egp", bufs=4))
        sm = ctx.enter_context(tc.tile_pool(name="sm", bufs=4))
        ep = ctx.enter_context(tc.tile_pool(name="ep", bufs=2))
        psuml = ctx.enter_context(tc.tile_pool(name="psuml", bufs=4, space="PSUM"))
        psumv = ctx.enter_context(tc.tile_pool(name="psumv", bufs=2, space="PSUM"))

        wt_sb = consts.tile([C, K], F16)
        nc.sync.dma_start(out=wt_sb[:], in_=wt_d[:])

        for n in range(S):
            x_sb = xpool.tile([C, HW_P], F16, tag="x_sb")
            half = HW_P // 2
            nc.sync.dma_start(out=x_sb[:, 0:half], in_=x_d[n][:, 0:half])
            nc.sync.dma_start(out=x_sb[:, half:], in_=x_d[n][:, half:])

            rhs_v = rvp.tile([TILE, RVW], F16, tag="rhs_v")
            rv = rhs_v[:, 0 : NTILES * VW].rearrange("p (t w) -> p t w", w=VW)
            hw = (NTILES // 2) * VW
            nc.scalar.dma_start(out=rhs_v[:, 0:hw], in_=xt_d[n][:, 0:hw])
            nc.scalar.dma_start(out=rhs_v[:, hw:], in_=xt_d[n][:, hw:])

            # ---- per-pixel L2 norm over C for the whole sample ----
            # sum(x^2) per pixel: bn_stats on DVE for most tiles, Square
            # with accum_out on ScalarE for the last few (engine balance);
            # bn_stats path: sum(x^2) = 64*(m_e^2 + m_o^2) + (64v_e + 64v_o)
            NACT = 0
            sumsq = sm.tile([TILE, NTILES, 1], F32, tag="sumsq")
            if NACT:
                sqjunk = sm.tile([TILE, TILE], F16, tag="sqjunk", bufs=1)
                for t in range(NTILES - NACT, NTILES):
                    nc.scalar.activation(
                        out=sqjunk[:],
                        in_=rv[:, t, 0:TILE],
                        func=mybir.ActivationFunctionType.Square,
                        accum_out=sumsq[:, t, :],
                    )
            stats = sm.tile([TILE, NTILES, 6], F32, tag="stats")
            for t in range(NTILES - NACT):
                nc.vector.bn_stats(stats[:, t, :], rv[:, t, 0:TILE])
            NB = NTILES - NACT
            msq = sm.tile([TILE, NB, 2], F32, tag="msq")
            nc.vector.tensor_mul(
                msq[:], stats[:, 0:NB, 1:5:3], stats[:, 0:NB, 1:5:3]
            )
            vsum = sm.tile([TILE, NB, 1], F32, tag="vsum")
            nc.vector.tensor_add(vsum[:], stats[:, 0:NB, 2:3], stats[:, 0:NB, 5:6])
            msum = sm.tile([TILE, NB, 1], F32, tag="msum")
            nc.vector.tensor_add(msum[:], msq[:, :, 0:1], msq[:, :, 1:2])
            nc.vector.scalar_tensor_tensor(
                out=sumsq[:, 0:NB, :], in0=msum[:], scalar=64.0, in1=vsum[:],
                op0=MUL, op1=ADD,
            )
            # rnorm = 1/sqrt(sumsq) = exp(-0.5*ln(sumsq))
            lns = sm.tile([TILE, NTILES, 1], F32, tag="lns")
            nc.scalar.activation(lns[:], sumsq[:], LN)
            rnorm = sm.tile([TILE, NTILES, 1], F32, tag="rnorm")
            nc.scalar.activation(rnorm[:], lns[:], EXP, scale=-0.5)

            vlad_ps = psumv.tile([TILE, VW], F32, tag="vlad_ps")

            # Software pipeline: step st's post-exp vector ops and vlad
            # matmuls are emitted during step st+1.
            pend = None  # (e_grp, first_tile_index)

            def flush_pending():
                nonlocal pend
                if pend is None:
                    return
                e_grp, t0 = pend
                denom = sm.tile([TILE, GST, 1], F32, tag="denom")
                nc.vector.reduce_sum(
                    denom[:],
                    e_grp[:, :, K : 2 * K],
                    axis=mybir.AxisListType.X,
                )
                with nc.allow_low_precision(reason="1/denom rounded to f32r"):
                    nc.vector.reciprocal(
                        out=rv[:, t0 : t0 + GST, TILE : TILE + 1], in_=denom[:]
                    )
                shat = sm.tile([TILE, GST, 1], F32, tag="shat")
                nc.vector.tensor_mul(
                    shat[:],
                    rnorm[:, t0 : t0 + GST, :],
                    rv[:, t0 : t0 + GST, TILE : TILE + 1],
                )
                nc.vector.tensor_mul(
                    e_grp[:, :, 0:K],
                    e_grp[:, :, K : 2 * K],
                    shat.broadcast_to([TILE, GST, K]),
                )
                for j in range(GST):
                    kk = t0 + j
                    nc.tensor.matmul(
                        vlad_ps[:],
                        e_grp[:, j, :],
                        rhs_v[:, kk * VW : kk * VW + VW],
                        start=(kk == 0),
                        stop=(kk == NTILES - 1),
                        skip_group_check=True,
                    )
                pend = None

            for st in range(NSTEP):
                lps = psuml.tile([TILE, GST, K], F32, tag="lps")
                for j in range(GST):
                    kk = st * GST + j
                    nc.tensor.matmul(
                        lps[:, j, :],
                        x_sb[:, kk * TILE : (kk + 1) * TILE],
                        wt_sb[:],
                        start=True,
                        stop=True,
                        skip_group_check=True,
                    )
                e_grp = egp.tile([TILE, GST, 2 * K], F16, tag="e_grp")
                for j in range(GST):
                    nc.scalar.activation(
                        out=e_grp[:, j, K : 2 * K],
                        in_=lps[:, j, 0:K],
                        func=EXP,
                        scale=rnorm[:, st * GST + j, :],
                    )
                flush_pending()
                pend = (e_grp, st * GST)

            flush_pending()

            vlad_sb = ep.tile([K, C], F32, tag="vlad_sb")
            nc.vector.tensor_copy(vlad_sb[:], vlad_ps[0:K, 0:C])
            s_sb = ep.tile([TILE, 1], F32, tag="s_sb")
            nc.scalar.copy(s_sb[K : 2 * K, :], vlad_ps[K : 2 * K, TILE : TILE + 1])
            nc.sync.dma_start(out=vladraw_d[n], in_=vlad_sb[:])
            nc.sync.dma_start(out=svec_d[n], in_=s_sb[K : 2 * K, :])

    nc.compile()
    return nc


def get_nc(S_=None):
    key = ("nc", S_ or S)
    if key not in _CACHE:
        _CACHE[key] = _build_nc(S_ or S)
    return _CACHE[key]


def make_xt2(x):
    """Full vlad-rhs image: xt2[n, p, t*129+c] = x[n, c, 128t+p] (fp16),
    zero 1/denom columns."""
    xt2 = np.zeros((x.shape[0], TILE, RVW), dtype=np.float16)
    view = xt2[:, :, 0 : NTILES * VW].reshape(x.shape[0], TILE, NTILES, VW)
    view[:, :, :, 0:TILE] = x.reshape(
        x.shape[0], C, NTILES, TILE
    ).transpose(0, 3, 2, 1)
    return xt2


def kernel(x, conv_w, conv_b, centroids, _run_kwargs=None):
    x = np.ascontiguousarray(np.asarray(x, dtype=np.float32)).reshape(N, C, HW_P)
    conv_w = np.asarray(conv_w, dtype=np.float32)
    conv_b = np.asarray(conv_b, dtype=np.float32)
    centroids = np.asarray(centroids, dtype=np.float32)
    assert np.all(conv_b == 0.0), "kernel assumes zero conv bias"

    nc = get_nc()
    wt = np.ascontiguousarray(conv_w.T).astype(np.float16)
    x16 = x.astype(np.float16)
    xt2 = make_xt2(x)
    in_maps = [
        {
            "x": x16[c * S : (c + 1) * S],
            "xt2": xt2[c * S : (c + 1) * S],
            "wt": wt,
        }
        for c in range(NCORES)
    ]
    res = run_bass_kernel_spmd(
        nc, in_maps, list(range(NCORES)), **(_run_kwargs or {})
    )
    vladraw = np.concatenate([r["vladraw"] for r in res.results], axis=0)  # [N,K,C]
    svec = np.concatenate([r["svec"] for r in res.results], axis=0)        # [N,K,1]

    vlad = vladraw - svec * centroids[None, :, :]      # [N,K,C]
    gmax = vlad.max(axis=1)                            # [N,C]
    global_fea = np.repeat(gmax, 2048 // C, axis=-1)   # [N,2048]
    _CACHE["last_results"] = res
    return global_fea.astype(np.float32), vlad.astype(np.float32)


if __name__ == "__main__":
    rng = np.random.default_rng(0)
    x = rng.standard_normal((N, C, 64, 64), dtype=np.float32)
    conv_w = rng.standard_normal((K, C), dtype=np.float32) * np.float32(
        np.sqrt(2.0 / K)
    )
    conv_b = np.zeros((K,), dtype=np.float32)
    centroids = rng.random((K, C), dtype=np.float32)
    gf, vl = kernel(x=x, conv_w=conv_w, conv_b=conv_b, centroids=centroids)
    print(gf.shape, vl.shape)


# revision 51
# speedup vs baseline: 1.0003x; 1.0003x over previous
"""NetVLAD Trainium2 Bass kernel.

Full-input contract: kernel(**inputs) takes the complete unsharded inputs
(x [32,128,64,64], conv_w [64,128], conv_b [64] (zeros), centroids [64,128])
and returns (global_fea [32,2048], vlad [32,64,128]) like the reference.

Sharding: data-parallel over N across 8 NeuronCores (4 samples per core);
params replicated; no cross-core communication.

x streams to each core in fp16 (11-bit mantissa, ~1e-4 quantization; the
whole-kernel output error vs the fp32 reference measures ~3e-5) in two
layouts so the PE never needs an on-chip transpose and no PSUM->SBUF copies
exist anywhere:
  - channel-major [C, P], the stationary operand of the 1x1-conv matmuls
  - a host-built "vlad rhs image": 129-wide slots per 128-pixel tile
    [XT_tile (pix x chan) | 1/denom column], DMA'd directly into the VLAD
    moving-operand buffer

Per sample:
  - per-tile DVE bn_stats on the pixel-major tiles -> per-pixel sum(x^2) ->
    one Ln + one Exp(scale=-0.5) per sample give rnorm = 1/||x||_2 (both in
    the natural_log_exp table set: exactly one ACT table load, see
    _patch_act_tables)
  - per 8-tile step: 8 logits matmuls into one PSUM bank, then per-tile
    ScalarE exp with per-partition scale=rnorm[tile]: e = exp(rnorm*logit)
    (softmax max-subtraction is skipped: |logit| <= max_k||w_k||_2 ~ 2.5)
  - one grouped DVE reduce gives the softmax denominators; a_hat =
    e * (rnorm/denom); both halves of lhsT = [a_hat | e] land in one tile
  - VLAD accumulated on PE across the 32 tiles of a sample:
      out[0:64, 0:128] = sum_p a_hat[p,k] * XT[p,c]   (vlad numerator)
      out[64:128, 128] = sum_p e[p,k] / denom[p] = s[k]  (softmax mass)
    The post-exp vector ops and vlad matmuls are software-pipelined one
    step behind the exp stage so no engine queue round-trips.
  - host finishes with exact fp32 ops: vlad = vladraw - s[:,None]*centroids,
    then the AdaptiveMaxPool2d((1,2048)) = max over K + 16x column repeat.
"""

import sys

sys.path.insert(0, "/opt/trn_rl_repo")

from contextlib import ExitStack

import numpy as np

import concourse.bacc as bacc
import concourse.tile as tile
from concourse import mybir
from concourse.bass_utils import run_bass_kernel_spmd

F32 = mybir.dt.float32
F32R = mybir.dt.float32r
F16 = mybir.dt.float16

N, C, HW_P, K = 32, 128, 4096, 64
NCORES = 8
S = N // NCORES            # samples per core
TILE = 128                 # pixels per tile
NTILES = HW_P // TILE      # 32
GST = 8                    # tiles per step (one PSUM bank of logits)
NSTEP = NTILES // GST      # 4
VW = TILE + 1              # per-tile slot width in the vlad rhs (129)
RVW = NTILES * VW          # vlad rhs width per sample

_CACHE = {}


def _patch_act_tables():
    """Force every activation into natural_log_exp_and_others (covers Exp,
    Ln, Copy, Identity) so exactly one ACT table load is emitted instead of
    thrashing between exp_and_others and natural_log_exp_and_others."""
    if _CACHE.get("act_patched"):
        return
    orig = bacc.get_activation_tables
    A = mybir.ActivationFunctionType
    strip = {A.Exp, A.Ln, A.Copy, A.Identity}
    keep = "natural_log_exp_and_others"

    def patched(arch):
        t = dict(orig(arch))
        if keep not in t:
            return t
        return {k: (v if k == keep else (set(v) - strip)) for k, v in t.items()}

    bacc.get_activation_tables = patched
    _CACHE["act_patched"] = True


def _build_nc(S=S):
    _patch_act_tables()
    nc = bacc.Bacc(
        "TRN2",
        target_bir_lowering=False,
        debug=False,
        num_devices=NCORES,
    )
    x_d = nc.dram_tensor("x", [S, C, HW_P], F16, kind="ExternalInput")
    # host-prepared vlad-rhs image: 129-wide slots [XT_tile | 0] plus a zero
    # tail, where xt2[n, p, t*129+c] = x[n, c, t*128+p] for c < 128
    xt_d = nc.dram_tensor("xt2", [S, TILE, RVW], F16, kind="ExternalInput")
    wt_d = nc.dram_tensor("wt", [C, K], F16, kind="ExternalInput")
    vladraw_d = nc.dram_tensor("vladraw", [S, K, C], F32, kind="ExternalOutput")
    svec_d = nc.dram_tensor("svec", [S, K, 1], F32, kind="ExternalOutput")

    EXP = mybir.ActivationFunctionType.Exp
    LN = mybir.ActivationFunctionType.Ln
    MUL = mybir.AluOpType.mult
    ADD = mybir.AluOpType.add

    with tile.TileContext(nc) as tc, ExitStack() as ctx:
        xpool = ctx.enter_context(tc.tile_pool(name="xpool", bufs=3))
        rvp = ctx.enter_context(tc.tile_pool(name="rvp", bufs=3))
        consts = ctx.enter_context(tc.tile_pool(name="consts", bufs=1))
        egp = ctx.enter_context(tc.tile_pool(name="egp", bufs=4))
        sm = ctx.enter_context(tc.tile_pool(name="sm", bufs=4))
        ep = ctx.enter_context(tc.tile_pool(name="ep", bufs=2))
        psuml = ctx.enter_context(tc.tile_pool(name="psuml", bufs=4, space="PSUM"))
        psumv = ctx.enter_context(tc.tile_pool(name="psumv", bufs=2, space="PSUM"))

        wt_sb = consts.tile([C, K], F16)
        nc.sync.dma_start(out=wt_sb[:], in_=wt_d[:])

        for n in range(S):
            x_sb = xpool.tile([C, HW_P], F16, tag="x_sb")
            half = HW_P // 2
            nc.sync.dma_start(out=x_sb[:, 0:half], in_=x_d[n][:, 0:half])
            nc.sync.dma_start(out=x_sb[:, half:], in_=x_d[n][:, half:])

            rhs_v = rvp.tile([TILE, RVW], F16, tag="rhs_v")
            rv = rhs_v[:, 0 : NTILES * VW].rearrange("p (t w) -> p t w", w=VW)
            hw = (NTILES // 2) * VW
            nc.scalar.dma_start(out=rhs_v[:, 0:hw], in_=xt_d[n][:, 0:hw])
            nc.scalar.dma_start(out=rhs_v[:, hw:], in_=xt_d[n][:, hw:])

            # ---- per-pixel L2 norm over C for the whole sample ----
            # sum(x^2) per pixel: bn_stats on DVE for most tiles, Square
            # with accum_out on ScalarE for the last few (engine balance);
            # bn_stats path: sum(x^2) = 64*(m_e^2 + m_o^2) + (64v_e + 64v_o)
            NACT = 0
            NB = NTILES - NACT
            NH = NTILES // 2
            sumsq = sm.tile([TILE, NTILES, 1], F32, tag="sumsq")
            if NACT:
                sqjunk = sm.tile([TILE, TILE], F16, tag="sqjunk", bufs=1)
                for t in range(NTILES - NACT, NTILES):
                    nc.scalar.activation(
                        out=sqjunk[:],
                        in_=rv[:, t, 0:TILE],
                        func=mybir.ActivationFunctionType.Square,
                        accum_out=sumsq[:, t, :],
                    )
            stats = sm.tile([TILE, NTILES, 6], F32, tag="stats")
            for t in range(NB):
                nc.vector.bn_stats(stats[:, t, :], rv[:, t, 0:TILE])
            msq = sm.tile([TILE, NB, 2], F32, tag="msq")
            nc.vector.tensor_mul(
                msq[:], stats[:, 0:NB, 1:5:3], stats[:, 0:NB, 1:5:3]
            )
            vsum = sm.tile([TILE, NB, 1], F32, tag="vsum")
            nc.vector.tensor_add(vsum[:], stats[:, 0:NB, 2:3], stats[:, 0:NB, 5:6])
            msum = sm.tile([TILE, NB, 1], F32, tag="msum")
            nc.vector.tensor_add(msum[:], msq[:, :, 0:1], msq[:, :, 1:2])
            nc.vector.scalar_tensor_tensor(
                out=sumsq[:, 0:NB, :], in0=msum[:], scalar=64.0, in1=vsum[:],
                op0=MUL, op1=ADD,
            )
            # rnorm = 1/sqrt(sumsq) = exp(-0.5*ln(sumsq)); split so the
            # early steps only depend on the DVE (bn_stats) portion
            lns = sm.tile([TILE, NTILES, 1], F32, tag="lns")
            rnorm = sm.tile([TILE, NTILES, 1], F32, tag="rnorm")
            nc.scalar.activation(lns[:, 0:NH, :], sumsq[:, 0:NH, :], LN)
            nc.scalar.activation(
                rnorm[:, 0:NH, :], lns[:, 0:NH, :], EXP, scale=-0.5
            )
            nc.scalar.activation(lns[:, NH:, :], sumsq[:, NH:, :], LN)
            nc.scalar.activation(
                rnorm[:, NH:, :], lns[:, NH:, :], EXP, scale=-0.5
            )

            vlad_ps = psumv.tile([TILE, VW], F32, tag="vlad_ps")

            # Software pipeline: step st's post-exp vector ops and vlad
            # matmuls are emitted during step st+1.
            pend = None  # (e_grp, first_tile_index)

            def flush_pending():
                nonlocal pend
                if pend is None:
                    return
                e_grp, t0 = pend
                denom = sm.tile([TILE, GST, 1], F32, tag="denom")
                nc.vector.reduce_sum(
                    denom[:],
                    e_grp[:, :, K : 2 * K],
                    axis=mybir.AxisListType.X,
                )
                with nc.allow_low_precision(reason="1/denom rounded to f32r"):
                    nc.vector.reciprocal(
                        out=rv[:, t0 : t0 + GST, TILE : TILE + 1], in_=denom[:]
                    )
                shat = sm.tile([TILE, GST, 1], F32, tag="shat")
                nc.vector.tensor_mul(
                    shat[:],
                    rnorm[:, t0 : t0 + GST, :],
                    rv[:, t0 : t0 + GST, TILE : TILE + 1],
                )
                nc.vector.tensor_mul(
                    e_grp[:, :, 0:K],
                    e_grp[:, :, K : 2 * K],
                    shat.broadcast_to([TILE, GST, K]),
                )
                for j in range(GST):
                    kk = t0 + j
                    nc.tensor.matmul(
                        vlad_ps[:],
                        e_grp[:, j, :],
                        rhs_v[:, kk * VW : kk * VW + VW],
                        start=(kk == 0),
                        stop=(kk == NTILES - 1),
                        skip_group_check=True,
                    )
                pend = None

            for st in range(NSTEP):
                lps = psuml.tile([TILE, GST, K], F32, tag="lps")
                for j in range(GST):
                    kk = st * GST + j
                    nc.tensor.matmul(
                        lps[:, j, :],
                        x_sb[:, kk * TILE : (kk + 1) * TILE],
                        wt_sb[:],
                        start=True,
                        stop=True,
                        skip_group_check=True,
                    )
                e_grp = egp.tile([TILE, GST, 2 * K], F16, tag="e_grp")
                for j in range(GST):
                    nc.scalar.activation(
                        out=e_grp[:, j, K : 2 * K],
                        in_=lps[:, j, 0:K],
                        func=EXP,
                        scale=rnorm[:, st * GST + j, :],
                    )
                flush_pending()
                pend = (e_grp, st * GST)

            flush_pending()

            vlad_sb = ep.tile([K, C], F32, tag="vlad_sb")
            nc.vector.tensor_copy(vlad_sb[:], vlad_ps[0:K, 0:C])
            s_sb = ep.tile([TILE, 1], F32, tag="s_sb")
            nc.scalar.copy(s_sb[K : 2 * K, :], vlad_ps[K : 2 * K, TILE : TILE + 1])
            nc.sync.dma_start(out=vladraw_d[n], in_=vlad_sb[:])
            nc.sync.dma_start(out=svec_d[n], in_=s_sb[K : 2 * K, :])

    nc.compile()
    return nc


def get_nc(S_=None):
    key = ("nc", S_ or S)
    if key not in _CACHE:
        _CACHE[key] = _build_nc(S_ or S)
    return _CACHE[key]


def make_xt2(x):
    """Full vlad-rhs image: xt2[n, p, t*129+c] = x[n, c, 128t+p] (fp16),
    zero 1/denom columns."""
    xt2 = np.zeros((x.shape[0], TILE, RVW), dtype=np.float16)
    view = xt2[:, :, 0 : NTILES * VW].reshape(x.shape[0], TILE, NTILES, VW)
    view[:, :, :, 0:TILE] = x.reshape(
        x.shape[0], C, NTILES, TILE
    ).transpose(0, 3, 2, 1)
    return xt2


def kernel(x, conv_w, conv_b, centroids, _run_kwargs=None):
    x = np.ascontiguousarray(np.asarray(x, dtype=np.float32)).reshape(N, C, HW_P)
    conv_w = np.asarray(conv_w, dtype=np.float32)
    conv_b = np.asarray(conv_b, dtype=np.float32)
    centroids = np.asarray(centroids, dtype=np.float32)
    assert np.all(conv_b == 0.0), "kernel assumes zero conv bias"

    nc = get_nc()
    wt = np.ascontiguousarray(conv_w.T).astype(np.float16)
    x16 = x.astype(np.float16)
    xt2 = make_xt2(x)
    in_maps = [
        {
            "x": x16[c * S : (c + 1) * S],
            "xt2": xt2[c * S : (c + 1) * S],
            "wt": wt,
        }
        for c in range(NCORES)
    ]
    res = run_bass_kernel_spmd(
        nc, in_maps, list(range(NCORES)), **(_run_kwargs or {})
    )
    vladraw = np.concatenate([r["vladraw"] for r in res.results], axis=0)  # [N,K,C]
    svec = np.concatenate([r["svec"] for r in res.results], axis=0)        # [N,K,1]

    vlad = vladraw - svec * centroids[None, :, :]      # [N,K,C]
    gmax = vlad.max(axis=1)                            # [N,C]
    global_fea = np.repeat(gmax, 2048 // C, axis=-1)   # [N,2048]
    _CACHE["last_results"] = res
    return global_fea.astype(np.float32), vlad.astype(np.float32)


if __name__ == "__main__":
    rng = np.random.default_rng(0)
    x = rng.standard_normal((N, C, 64, 64), dtype=np.float32)
    conv_w = rng.standard_normal((K, C), dtype=np.float32) * np.float32(
        np.sqrt(2.0 / K)
    )
    conv_b = np.zeros((K,), dtype=np.float32)
    centroids = rng.random((K, C), dtype=np.float32)
    gf, vl = kernel(x=x, conv_w=conv_w, conv_b=conv_b, centroids=centroids)
    print(gf.shape, vl.shape)
